# revision 13
# baseline (speedup 1.0000x reference)
"""Causal multi-head attention (B=2,T=2048,C=1024,H=16,Ca=64) on 8 trn2 cores.

Sharding: the 32 (batch, head) pairs are split across 8 cores — core c gets
batch b = c//4 and heads [4g, 4g+4) where g = c%4.  Each core computes its
heads' attention plus the partial output projection through its 256-row slice
of w_o; the host sums the 4 partials per batch.

Per-core layouts (everything keeps the contraction dim on partitions):
  xT   [8,128,2048]  x[b].T c-chunked
  wq/wk[2,8,128,128] per head-pair, per c-chunk, cols = [h0 64 | h1 64]
  wv   [8,128,256]   4 heads concatenated
  wo   [2,128,1024]  rows 256g..256g+256 of w_o, c_local-chunked
  out  [16,128,1024] partial output, t-blocked

On-chip: Q^T,K^T [128(2 heads),2048]; V natural [s,a] with a ones column
appended so the A@V matmul also emits the softmax row-sums l[t]; scores are
computed transposed (S^T[s,t]) so softmax needs no cross-partition reduction
and no max-subtraction (logits are bounded: |s*scale| < ~4).
"""

import math
import sys

import numpy as np

for _p in ("/opt/trn_rl_repo",):
    if _p not in sys.path:
        sys.path.insert(0, _p)

import concourse.bass as bass
from concourse import bacc
import concourse.mybir as mybir
from concourse.bass import ts
from concourse.tile import TileContext
from concourse.bass_utils import run_bass_kernel_spmd
from contextlib import ExitStack

F32 = mybir.dt.float32
F32R = mybir.dt.float32r
AF = mybir.ActivationFunctionType

B, T, C = 2, 2048, 1024
H, CA = 16, 64
SCALE = 1.0 / math.sqrt(CA)
NCORES = 8
HPC = 4          # heads per core
TB = T // 128    # 16 t-blocks of 128
TC = T // 512    # 4 t-chunks of 512
CK = C // 128    # 8 c-chunks




def build_nc():
    nc = bacc.Bacc()
    xT = nc.declare_dram_parameter("xT", [CK, 128, T], F32R, isOutput=False)
    wq = nc.declare_dram_parameter("wq", [2, CK, 128, 128], F32R, isOutput=False)
    wk = nc.declare_dram_parameter("wk", [2, CK, 128, 128], F32R, isOutput=False)
    wv = nc.declare_dram_parameter("wv", [CK, 128, 2 * 128], F32R, isOutput=False)
    wo = nc.declare_dram_parameter("wo", [2, 128, C], F32R, isOutput=False)
    mask_d = nc.declare_dram_parameter("mask", [128, 4, 512], F32R, isOutput=False)
    ones_d = nc.declare_dram_parameter("ones", [128, 64], F32R, isOutput=False)
    out = nc.declare_dram_parameter("out", [TB, 128, C], F32, isOutput=True)

    with TileContext(nc) as tc, ExitStack() as ctx:
        const = ctx.enter_context(tc.tile_pool(name="const", bufs=1))
        persist = ctx.enter_context(tc.tile_pool(name="persist", bufs=1))

        # 0/1 causal masks for the 4 diagonal-band shifts (S^T layout [s,t]):
        # keep (1.0) where 128*d + p <= f, else 0.  Host-computed.
        mask = const.tile([128, 4, 512], F32R)
        nc.scalar.dma_start(mask[:], mask_d[:])
        ones_sb = const.tile([128, 64], F32R)
        nc.gpsimd.dma_start(ones_sb[:], ones_d[:])
        ones1 = ones_sb[0:1, :]

        q_sb = [persist.tile([128, T], F32R, tag=f"q{p}", name=f"q{p}") for p in range(2)]
        k_sb = [persist.tile([128, T], F32R, tag=f"k{p}", name=f"k{p}") for p in range(2)]
        # V natural [s,a] per head, t-blocked, with ones column at a=64
        v_sb = persist.tile([128, HPC, TB, 65], F32R, tag="v")
        nc.sync.dma_start(
            v_sb[:, :, :, 64],
            ones_d[:].rearrange("p (h b) -> p h b", h=HPC),
        )
        y_sb = [persist.tile([128, T], F32R, tag=f"y{p}", name=f"y{p}") for p in range(2)]
        wo_sb = persist.tile([128, 2, C], F32R, tag="wo")
        for cl in range(2):
            nc.gpsimd.dma_start(wo_sb[:, cl, :], wo[cl])

        # ---------------- Phase B/C: projections ----------------
        with ExitStack() as pbc:
            xw = pbc.enter_context(tc.tile_pool(name="xw", bufs=1))
            ps_qk = pbc.enter_context(tc.tile_pool(name="ps_qk", bufs=4, space="PSUM"))
            ps_v = pbc.enter_context(tc.tile_pool(name="ps_v", bufs=3, space="PSUM"))

            xT_sb = xw.tile([128, CK, T], F32R, tag="xT")
            wq_sb = xw.tile([128, 2, CK, 128], F32R, tag="wq")
            wk_sb = xw.tile([128, 2, CK, 128], F32R, tag="wk")
            wv_sb = xw.tile([128, CK, 256], F32R, tag="wv")
            # weights for pair 0 first (first matmuls need them), x chunks
            # round-robined over issuing engines so queues run in parallel
            engs = [nc.sync, nc.scalar, nc.gpsimd]
            nc.sync.dma_start(xT_sb[:, 0, :], xT[0])
            nc.scalar.dma_start(wq_sb[:, 0, 0, :], wq[0, 0])
            for ck in range(1, CK):
                engs[ck % 3].dma_start(wq_sb[:, 0, ck, :], wq[0, ck])
            for ck in range(1, CK):
                engs[ck % 3].dma_start(xT_sb[:, ck, :], xT[ck])
            for ck in range(CK):
                engs[(ck + 1) % 3].dma_start(wk_sb[:, 0, ck, :], wk[0, ck])
                engs[(ck + 2) % 3].dma_start(wq_sb[:, 1, ck, :], wq[1, ck])
                engs[ck % 3].dma_start(wk_sb[:, 1, ck, :], wk[1, ck])
                engs[(ck + 1) % 3].dma_start(wv_sb[:, ck, :], wv[ck])

            # Q^T / K^T: [128(2 heads), T]
            for p in range(2):
                for w_s, dst in ((wq_sb, q_sb), (wk_sb, k_sb)):
                    pst = [ps_qk.tile([128, 512], F32, tag="qk", name="qkps") for _ in range(TC)]
                    for ck in range(CK):
                        for tcn in range(TC):
                            nc.tensor.matmul(
                                pst[tcn][:],
                                lhsT=(w_s[:, p, ck, :]),
                                rhs=(xT_sb[:, ck, ts(tcn, 512)]),
                                start=(ck == 0), stop=(ck == CK - 1),
                            )
                    for tcn in range(TC):
                        nc.vector.tensor_copy(dst[p][:, ts(tcn, 512)], pst[tcn][:])

            # V natural: [s(=t) blocks, 4*64]
            for tb in range(TB):
                vps = ps_v.tile([128, 256], F32, tag="v")
                for ck in range(CK):
                    nc.tensor.matmul(
                        vps[:],
                        lhsT=(xT_sb[:, ck, ts(tb, 128)]),
                        rhs=(wv_sb[:, ck, :]),
                        start=(ck == 0), stop=(ck == CK - 1),
                    )
                nc.vector.tensor_copy(
                    v_sb[:, :, tb, 0:64],
                    vps[:].rearrange("p (h a) -> p h a", h=HPC),
                )

        # ---------------- Phase D + E: attention and output projection ----
        # tcn-outer so the projection for finished t-chunks overlaps attention
        with ExitStack() as pd:
            pp = pd.enter_context(tc.tile_pool(name="pp", bufs=10))
            sm = pd.enter_context(tc.tile_pool(name="sm", bufs=4))
            ob = pd.enter_context(tc.tile_pool(name="ob", bufs=3))
            ps_s = pd.enter_context(tc.tile_pool(name="ps_s", bufs=2, space="PSUM"))
            ps_y = pd.enter_context(tc.tile_pool(name="ps_y", bufs=2, space="PSUM"))
            ps_o = pd.enter_context(tc.tile_pool(name="ps_o", bufs=2, space="PSUM"))

            def proj_block(tb):
                ot = ob.tile([128, C], F32, tag="o", name="ot")
                for cc in range(2):
                    ops_ = ps_o.tile([128, 512], F32, tag="o", name="ops")
                    for cl in range(2):
                        nc.tensor.matmul(
                            ops_[:],
                            lhsT=(y_sb[cl][:, ts(tb, 128)]),
                            rhs=(wo_sb[:, cl, ts(cc, 512)]),
                            start=(cl == 0), stop=(cl == 1),
                        )
                    nc.vector.tensor_copy(ot[:, ts(cc, 512)], ops_[:])
                nc.sync.dma_start(out[tb], ot[:])

            for tcn in range(TC):
                nsb = 4 * tcn + 4
                for p in range(2):
                    for hl in range(2):
                        h = 2 * p + hl
                        b0 = 64 * hl
                        yps = ps_y.tile([128, 512], F32, tag="y", name="yps")
                        for sb2 in range(0, nsb, 2):
                            sps = ps_s.tile([128, 1024], F32, tag="s", name="sps")
                            for j in range(2):
                                nc.tensor.matmul(
                                    sps[:, ts(j, 512)],
                                    lhsT=(k_sb[p][b0:b0 + 64, ts(sb2 + j, 128)]),
                                    rhs=(q_sb[p][b0:b0 + 64, ts(tcn, 512)]),
                                    start=True, stop=True,
                                )
                            pb = pp.tile([128, 1024], F32R, tag="pb", name="pb")
                            nc.scalar.activation(pb[:], sps[:], AF.Exp, scale=SCALE)
                            for j in range(2):
                                d = sb2 + j - 4 * tcn
                                if d >= 0:
                                    w = 128 * (d + 1)
                                    o = 512 * j
                                    nc.vector.tensor_mul(
                                        pb[:, o:o + w], pb[:, o:o + w],
                                        mask[:, d, :w])
                            for j in range(2):
                                nc.tensor.matmul(
                                    yps[0:65, :],
                                    lhsT=(v_sb[:, h, sb2 + j, :]),
                                    rhs=(pb[:, ts(j, 512)]),
                                    start=(sb2 + j == 0), stop=(sb2 + j == nsb - 1),
                                )
                        # normalize: y /= l (l = row 64 of yps)
                        lrow = sm.tile([1, 512], F32R, tag="l", name="lrow")
                        nc.vector.tensor_copy(lrow[:], yps[64:65, :])
                        bps = ps_o.tile([128, 512], F32, tag="o", name="bps")
                        nc.tensor.matmul(
                            bps[0:64, :], lhsT=(ones1[:]), rhs=(lrow[:]),
                            start=True, stop=True,
                        )
                        rb = sm.tile([64, 512], F32, tag="r", name="rb")
                        nc.vector.reciprocal(rb[:], bps[0:64, :])
                        nc.vector.tensor_mul(
                            y_sb[p][b0:b0 + 64, ts(tcn, 512)],
                            yps[0:64, :], rb[:],
                        )
                # project the 4 t-blocks of this finished chunk
                for tb in range(4 * tcn, 4 * tcn + 4):
                    proj_block(tb)

    nc.compile()
    return nc


_NC = None


def _get_nc():
    global _NC
    if _NC is None:
        _NC = build_nc()
    return _NC


def _mask_arr():
    p = np.arange(128)[:, None, None]
    d = np.arange(4)[None, :, None]
    f = np.arange(512)[None, None, :]
    return np.ascontiguousarray((128 * d + p <= f).astype(np.float32))


def make_in_maps(x, w_q, w_k, w_v, w_o):
    x = np.asarray(x, dtype=np.float32)
    w_q = np.asarray(w_q, dtype=np.float32)
    w_k = np.asarray(w_k, dtype=np.float32)
    w_v = np.asarray(w_v, dtype=np.float32)
    w_o = np.asarray(w_o, dtype=np.float32)
    in_maps = []
    for c in range(NCORES):
        b, g = c // 4, c % 4
        hs = [4 * g + i for i in range(HPC)]
        xT = np.ascontiguousarray(x[b].T).reshape(CK, 128, T)
        wq_a = np.stack([
            np.concatenate([w_q[hs[2 * p]], w_q[hs[2 * p + 1]]], axis=1).reshape(CK, 128, 128)
            for p in range(2)
        ])
        wk_a = np.stack([
            np.concatenate([w_k[hs[2 * p]], w_k[hs[2 * p + 1]]], axis=1).reshape(CK, 128, 128)
            for p in range(2)
        ])
        wv_a = np.concatenate([w_v[h] for h in hs], axis=1).reshape(CK, 128, 256)
        wo_a = w_o[256 * g:256 * (g + 1)].reshape(2, 128, C)
        in_maps.append(dict(
            mask=_mask_arr(),
            ones=np.ones((128, 64), np.float32),
            xT=np.ascontiguousarray(xT),
            wq=np.ascontiguousarray(wq_a),
            wk=np.ascontiguousarray(wk_a),
            wv=np.ascontiguousarray(wv_a),
            wo=np.ascontiguousarray(wo_a),
        ))
    return in_maps


def gather_out(results):
    acc = [np.zeros((T, C), np.float64) for _ in range(B)]
    for c in range(NCORES):
        acc[c // 4] += results[c]["out"].reshape(T, C).astype(np.float64)
    return np.stack([a.astype(np.float32) for a in acc])


def run(x, w_q, w_k, w_v, w_o, trace=False, **spmd_kwargs):
    nc = _get_nc()
    in_maps = make_in_maps(x, w_q, w_k, w_v, w_o)
    res = run_bass_kernel_spmd(nc, in_maps, list(range(NCORES)), trace=trace,
                               **spmd_kwargs)
    return gather_out(res.results), res


def kernel(x, w_q, w_k, w_v, w_o):
    out, _ = run(x, w_q, w_k, w_v, w_o)
    return out
